# revision 2
# baseline (speedup 1.0000x reference)
"""Locally-connected conv (BioConvolution) Trainium2 kernel.

Problem: Z[n,p,o] = relu(sum_{ijc} patch[n,p,i,j,c] * filt[p,i,j,c,o] + bias[o])
  X: (32,128,128,32) f32, filters: (1024,4,4,32,32) f32, bias: (32,)
  out: (32,32,32,32) f32.   FH=FW=4 non-overlapping patches, P=1024.

Sharding: patch-parallel over P across 8 cores. Core k owns patches
[128k,128k+128) == image rows [16k,16k+16). Each core touches only its own
X rows and filters: 16.8 MB in + 0.5 MB out per core — the true memory
roofline (~48 us at 358 GB/s per-core HBM; no operand is reused anywhere).

Host-side marshaling (part of sharding): the contraction axis must sit on
SBUF partitions for the PE, so X is pre-arranged per-core into
  xt[r, p, q, b] = X[b, 16k+4*pr+q, 4*pc+j, c]   (r = j*32+c, p = pr*32+pc)
and the filters into the matching ft[r, p, q, o]; both are packed into one
r-major array xf (data cols 0:32, filter cols 32:64) so every HBM->SBUF
DMA moves 128 partitions x multi-KB contiguous runs at line rate.

Device kernel (identical SPMD program on 8 cores), shipped variant fp32r:
  - All input loads issue from the sync engine's single HWDGE FIFO:
    strictly in-order chunk completions (concurrently-armed queues would
    round-robin and synchronize their completions, starving the PE), with
    a graduated [2,2,4] head so the first matmul starts early and a [4,4]
    tail to shorten the final dependency chain. bufs=8 double-buffering.
  - Per patch: 4 accumulating float32r matmuls (K=128, M=32 fout, N=32
    batch) — single-pass fp32 (~tf32 precision, rel err ~1.5e-4, half the
    PE instruction stream of true fp32 which lowers to LO/HI pairs).
    fp32r requires PSUM base partition 0, so 8 patches pack side-by-side
    along the free axis of one PSUM bank [32, 8x32].
  - ScalarE applies bias+ReLU per PSUM bank into an SBUF staging buffer;
    output stores ride ScalarE's own HWDGE ring LAGGED two groups behind
    the ACT stream, so their dependencies are long complete and they can
    never head-of-line block either the load FIFO or the ACT stream
    (gpsimd/SWDGE stores were tried and added multi-us Q7 drain jitter).
  - Two 4-patch mini-groups at the end halve the final
    load->matmul->ACT->store dependency chain.
Measured: ~62-66 us NEFF exec across runs (~±2 us device jitter), vs a
~48 us pure-traffic roofline at the 358 GB/s per-core HBM wall; ~8.7 us
is fixed engine-boot/Tile-preamble before the first DMA packet can flow,
~4 us is the unavoidable tail (final chain + store completion + Tile
drain barrier).
"""

import numpy as np

N, H, W, C = 32, 128, 128, 32
FH = FW = 4
FOUT = 32
NCORES = 8
PL = 128          # patches per core
NQ = 4            # K-chunks per patch (512 / 128)
KR = 128          # contraction rows per chunk (SBUF partitions)
NG = PL // 4      # 4-patch groups per core

_CACHE = {}


def _build_module(bufs=6, out_splits=8, mm_dtype="float32"):
    from concourse import bacc, tile, mybir

    nc = bacc.Bacc("TRN2", target_bir_lowering=False, debug=False, enable_asserts=False)
    dt = mybir.dt.float32
    mdt = getattr(mybir.dt, mm_dtype)
    # xf packs data and filters: [..., 0:32] = batch cols, [..., 32:64] = fout
    xf = nc.dram_tensor("xf", [KR, PL, NQ, N + FOUT], mdt, kind="ExternalInput").ap()
    bt = nc.dram_tensor("bt", [KR, 1], dt, kind="ExternalInput").ap()
    out = nc.dram_tensor("out", [KR, NG, N], dt, kind="ExternalOutput").ap()

    # Graduated chunk sizes (in patches): small first chunks so the first
    # matmul isn't gated on a full-size load sharing bandwidth round-robin.
    sizes = [2, 2, 4]
    rest = PL - sum(sizes)
    sizes += [8] * (rest // 8)
    assert sum(sizes) == PL
    GSPLIT = NG // out_splits
    relu = mybir.ActivationFunctionType.Relu

    with tile.TileContext(nc) as tc:
        with (
            tc.tile_pool(name="xfpool", bufs=bufs) as xfpool,
            tc.tile_pool(name="psum", bufs=8, space="PSUM") as psum,
            tc.tile_pool(name="misc", bufs=1) as misc,
        ):
            bias_t = misc.tile([KR, 1], dt)
            nc.sync.dma_start(bias_t[:], bt[:])
            staging = misc.tile([KR, NG, N], dt)

            p0 = 0
            for ch, PC in enumerate(sizes):
                xtile = xfpool.tile([KR, PC, NQ, N + FOUT], mdt, tag="xf")
                sl = slice(p0, p0 + PC)
                eng = nc.sync if ch % 2 == 0 else nc.scalar
                eng.dma_start(xtile[:], xf[:, sl, :, :])
                for g in range(PC // 2):
                    gg = (p0 + g * 2) // 4       # psum group id (2 patches/iter)
                    half = (p0 + g * 2) % 4      # 0 or 2: which half of the group
                    if half == 0:
                        ptile = psum.tile([KR, N], dt, tag="ps")
                    for s2 in range(2):
                        s = half + s2
                        p = g * 2 + s2
                        for q in range(NQ):
                            nc.tensor.matmul(
                                ptile[32 * s : 32 * s + 32, :],
                                xtile[:, p, q, N : N + FOUT],  # lhsT [128,32(o)]
                                xtile[:, p, q, 0:N],           # rhs  [128,32(b)]
                                start=(q == 0),
                                stop=(q == NQ - 1),
                                tile_position=(0, 32 * s),
                            )
                    if half == 2:
                        nc.scalar.activation(
                            staging[:, gg, :], ptile[:], relu, bias=bias_t[:]
                        )
                        if (gg + 1) % GSPLIT == 0:
                            osl = slice(gg + 1 - GSPLIT, gg + 1)
                            oeng = nc.sync if gg + 1 == NG else nc.gpsimd
                            oeng.dma_start(out[:, osl, :], staging[:, osl, :])
                p0 += PC
    nc.compile()
    return nc


def _build_module_r(bufs=8):
    """float32r variant: single-pass fp32 matmuls (tf32-ish precision),
    PSUM packing along the free axis (8 patches per bank) since fp32r
    requires dst base partition 0. Half the PE instruction stream of the
    fp32 variant -> fewer IRAM paging stalls."""
    from concourse import bacc, tile, mybir

    nc = bacc.Bacc("TRN2", target_bir_lowering=False, debug=False, enable_asserts=False)
    dt = mybir.dt.float32
    mdt = mybir.dt.float32r
    SG = 8                      # patches per PSUM super-group
    NSG = PL // SG              # 16
    xf = nc.dram_tensor("xf", [KR, PL, NQ, N + FOUT], mdt, kind="ExternalInput").ap()
    bt = nc.dram_tensor("bt", [FOUT, 1], dt, kind="ExternalInput").ap()
    out = nc.dram_tensor("out", [FOUT, PL, N], dt, kind="ExternalOutput").ap()

    # Graduated [2,2,4] head (earliest first matmul; measured tightest
    # variance) and a [4,4] tail that halves the final
    # load->matmul->ACT->store chain.
    sizes = [2, 2, 4] + [8] * ((PL - 16) // 8) + [4, 2, 2]
    assert sum(sizes) == PL
    # PSUM eviction groups: 8-patch banks, except two 4-patch mini-groups
    # at the end so the last matmul->ACT->store chain is half as long.
    groups = [(g * SG, SG) for g in range(NSG - 1)] + [(PL - 8, 4), (PL - 4, 4)]
    gof = {}
    for gi, (s0, gsz) in enumerate(groups):
        for i in range(gsz):
            gof[s0 + i] = (gi, i)
    relu = mybir.ActivationFunctionType.Relu

    with tile.TileContext(nc) as tc:
        with (
            tc.tile_pool(name="xfpool", bufs=bufs) as xfpool,
            tc.tile_pool(name="psum", bufs=6, space="PSUM") as psum,
            tc.tile_pool(name="misc", bufs=1) as misc,
        ):
            # bias rides the scalar ring so it doesn't burn sync's first
            # DMA slot (~0.7 us of stream start).
            bias_t = misc.tile([FOUT, 1], dt)
            nc.scalar.dma_start(bias_t[:], bt[:])
            staging = misc.tile([FOUT, PL, N], dt)

            p0 = 0
            ptile = None
            for ch, PC in enumerate(sizes):
                xtile = xfpool.tile([KR, PC, NQ, N + FOUT], mdt, tag="xf")
                # All loads on sync's single HWDGE FIFO: strictly in-order
                # completions. (Arming chunk 0 on the scalar ring was tried
                # and is bimodal: when sync's big queue gets ahead, chunk 0
                # drains at round-robin half-rate and the in-order PE
                # consumption slips ~8 us.)
                nc.sync.dma_start(xtile[:], xf[:, p0 : p0 + PC, :, :])
                for pl in range(PC):
                    p = p0 + pl
                    gi, i = gof[p]
                    s0, gsz = groups[gi]
                    if i == 0:
                        ptile = psum.tile([FOUT, SG, N], dt, tag="ps")
                    for q in range(NQ):
                        nc.tensor.matmul(
                            ptile[:, i, :],
                            xtile[:, pl, q, N : N + FOUT],  # lhsT [128,32(o)]
                            xtile[:, pl, q, 0:N],           # rhs  [128,32(b)]
                            start=(q == 0),
                            stop=(q == NQ - 1),
                        )
                    if i == gsz - 1:
                        nc.scalar.activation(
                            staging[:, s0 : s0 + gsz, :],
                            ptile[:, :gsz, :],
                            relu,
                            bias=bias_t[:],
                        )
                        # Stores also ride the scalar ring, LAGGED two groups
                        # behind the ACT stream: their ACT dependency is long
                        # complete, so they never stall scalar (and the sync
                        # load ring is untouched). The final two stores are
                        # pure program-order after the last ACT.
                        if gi == len(groups) - 1:
                            a = groups[gi - 2][0]
                            nc.scalar.dma_start(
                                out[:, a:s0, :], staging[:, a:s0, :]
                            )
                            nc.scalar.dma_start(
                                out[:, s0:PL, :], staging[:, s0:PL, :]
                            )
                        elif gi % 2 == 1 and gi >= 3:
                            a = groups[gi - 3][0]
                            b = groups[gi - 1][0]
                            nc.scalar.dma_start(
                                out[:, a:b, :], staging[:, a:b, :]
                            )
                p0 += PC
    nc.compile()
    return nc


def _build_module_h(bufs=8, out_dt="float16"):
    """fp16 variant: inputs marshaled to float16 on host (HBM traffic
    halves vs fp32 — this problem is memory-bound with zero operand
    reuse), matmuls run 1 cycle/row on the PE (vs 4 for fp32r at free
    dim 32 < 256) with fp32 PSUM accumulation. rel err ~2e-4, far under
    the 2e-2 gate. Same stream structure as the fp32r variant."""
    from concourse import bacc, tile, mybir

    nc = bacc.Bacc("TRN2", target_bir_lowering=False, debug=False, enable_asserts=False)
    dt = mybir.dt.float32
    mdt = mybir.dt.float16
    odt = getattr(mybir.dt, out_dt)
    SG = 8                      # patches per PSUM super-group
    NSG = PL // SG              # 16
    xf = nc.dram_tensor("xf", [KR, PL, NQ, N + FOUT], mdt, kind="ExternalInput").ap()
    bt = nc.dram_tensor("bt", [FOUT, 1], dt, kind="ExternalInput").ap()
    out = nc.dram_tensor("out", [FOUT, PL, N], odt, kind="ExternalOutput").ap()

    sizes = [2, 2, 4] + [8] * ((PL - 16) // 8) + [4, 2, 2]
    assert sum(sizes) == PL
    groups = [(g * SG, SG) for g in range(NSG - 1)] + [(PL - 8, 4), (PL - 4, 4)]
    gof = {}
    for gi, (s0, gsz) in enumerate(groups):
        for i in range(gsz):
            gof[s0 + i] = (gi, i)
    relu = mybir.ActivationFunctionType.Relu

    with tile.TileContext(nc) as tc:
        with (
            tc.tile_pool(name="xfpool", bufs=bufs) as xfpool,
            tc.tile_pool(name="psum", bufs=6, space="PSUM") as psum,
            tc.tile_pool(name="misc", bufs=1) as misc,
        ):
            bias_t = misc.tile([FOUT, 1], dt)
            nc.scalar.dma_start(bias_t[:], bt[:])
            staging = misc.tile([FOUT, PL, N], odt)

            p0 = 0
            ptile = None
            for ch, PC in enumerate(sizes):
                xtile = xfpool.tile([KR, PC, NQ, N + FOUT], mdt, tag="xf")
                nc.sync.dma_start(xtile[:], xf[:, p0 : p0 + PC, :, :])
                for pl in range(PC):
                    p = p0 + pl
                    gi, i = gof[p]
                    s0, gsz = groups[gi]
                    if i == 0:
                        ptile = psum.tile([FOUT, SG, N], dt, tag="ps")
                    for q in range(NQ):
                        nc.tensor.matmul(
                            ptile[:, i, :],
                            xtile[:, pl, q, N : N + FOUT],  # lhsT [128,32(o)]
                            xtile[:, pl, q, 0:N],           # rhs  [128,32(b)]
                            start=(q == 0),
                            stop=(q == NQ - 1),
                        )
                    if i == gsz - 1:
                        nc.scalar.activation(
                            staging[:, s0 : s0 + gsz, :],
                            ptile[:, :gsz, :],
                            relu,
                            bias=bias_t[:],
                        )
                        if gi == len(groups) - 1:
                            a = groups[gi - 2][0]
                            nc.scalar.dma_start(
                                out[:, a:s0, :], staging[:, a:s0, :]
                            )
                            nc.scalar.dma_start(
                                out[:, s0:PL, :], staging[:, s0:PL, :]
                            )
                        elif gi % 2 == 1 and gi >= 3:
                            a = groups[gi - 3][0]
                            b = groups[gi - 1][0]
                            nc.scalar.dma_start(
                                out[:, a:b, :], staging[:, a:b, :]
                            )
                p0 += PC
    nc.compile()
    return nc


def _get_module():
    if "nc" not in _CACHE:
        _CACHE["nc"] = _build_module()
    return _CACHE["nc"]


def _marshal(X, filters, bias):
    """Shard + lay out full inputs into per-core device arrays."""
    X = np.ascontiguousarray(np.asarray(X, dtype=np.float32))
    filters = np.ascontiguousarray(np.asarray(filters, dtype=np.float32))
    bias = np.asarray(bias, dtype=np.float32)

    # X: (b, core, pr, i, pc, j, c) -> (core, j, c, pr, pc, i, b)
    xv = X.reshape(N, NCORES, 4, FH, 32, FW, C)
    xt = xv.transpose(1, 5, 6, 2, 4, 3, 0).reshape(NCORES, KR, PL, NQ, N)
    # filters: (core, p, i, j, c, o) -> (core, j, c, p, i, o)
    fv = filters.reshape(NCORES, PL, FH, FW, C, FOUT)
    ft = fv.transpose(0, 3, 4, 1, 2, 5).reshape(NCORES, KR, PL, NQ, FOUT)
    xfa = np.ascontiguousarray(np.concatenate([xt, ft], axis=4))
    bt = np.ascontiguousarray(np.tile(bias, 4).reshape(KR, 1))
    return xfa, bt


def _assemble(outs):
    """Per-core out [128=(s,o), NG, N] -> full (N, 32, 32, FOUT)."""
    z = np.stack(outs)                                  # (core, (s,o), g, b)
    z = z.reshape(NCORES, 4, FOUT, NG, N)               # (core, s, o, g, b)
    z = z.transpose(4, 0, 3, 1, 2)                      # (b, core, g, s, o)
    z = z.reshape(N, NCORES, PL, FOUT)                  # p_loc = 4*g + s
    z = z.reshape(N, NCORES * 4, 32, FOUT)              # (b, pr_glob, pc, o)
    return np.ascontiguousarray(z)


def _assemble_r(outs):
    """Per-core out [FOUT, PL, N] -> full (N, 32, 32, FOUT)."""
    z = np.stack(outs)                                  # (core, o, p, b)
    z = z.transpose(3, 0, 2, 1)                         # (b, core, p, o)
    return np.ascontiguousarray(z.reshape(N, 32, 32, FOUT))


LAST_RESULT = None
VARIANT = "fp32r"


def kernel(X, filters, bias):
    global LAST_RESULT
    from concourse import bass_utils
    from concourse.bass_utils import run_bass_kernel_spmd

    # If tracing is enabled in the environment, keep the artifact upload
    # local so a missing bucket can't fail the run.
    bass_utils.upload_artifacts = lambda tmpdir: f"local://{tmpdir}"

    if "nc" not in _CACHE:
        _CACHE["nc"] = (
            _build_module_r() if VARIANT == "fp32r" else _build_module()
        )
    nc = _CACHE["nc"]
    xfa, bt = _marshal(X, filters, bias)
    if VARIANT == "fp32r":
        bt = np.ascontiguousarray(bt[:FOUT])
    in_maps = [{"xf": xfa[k], "bt": bt} for k in range(NCORES)]
    res = run_bass_kernel_spmd(nc, in_maps, core_ids=list(range(NCORES)))
    LAST_RESULT = res
    outs = [res.results[k]["out"] for k in range(NCORES)]
    return _assemble_r(outs) if VARIANT == "fp32r" else _assemble(outs)



# revision 4
# speedup vs baseline: 1.6358x; 1.6358x over previous
"""Locally-connected conv (BioConvolution) Trainium2 kernel.

Problem: Z[n,p,o] = relu(sum_{ijc} patch[n,p,i,j,c] * filt[p,i,j,c,o] + bias[o])
  X: (32,128,128,32) f32, filters: (1024,4,4,32,32) f32, bias: (32,)
  out: (32,32,32,32) f32.   FH=FW=4 non-overlapping patches, P=1024.

Sharding: patch-parallel over P across 8 cores. Core k owns patches
[128k,128k+128) == image rows [16k,16k+16). Each core touches only its own
X rows and filters: 16.8 MB in + 0.5 MB out per core — the true memory
roofline (~48 us at 358 GB/s per-core HBM; no operand is reused anywhere).

Host-side marshaling (part of sharding): the contraction axis must sit on
SBUF partitions for the PE, so X is pre-arranged per-core into
  xt[r, p, q, b] = X[b, 16k+4*pr+q, 4*pc+j, c]   (r = j*32+c, p = pr*32+pc)
and the filters into the matching ft[r, p, q, o]; both are packed into one
r-major array xf (data cols 0:32, filter cols 32:64) so every HBM->SBUF
DMA moves 128 partitions x multi-KB contiguous runs at line rate.

Device kernel (identical SPMD program on 8 cores), shipped variant fp32r:
  - All input loads issue from the sync engine's single HWDGE FIFO:
    strictly in-order chunk completions (concurrently-armed queues would
    round-robin and synchronize their completions, starving the PE), with
    a graduated [2,2,4] head so the first matmul starts early and a [4,4]
    tail to shorten the final dependency chain. bufs=8 double-buffering.
  - Per patch: 4 accumulating float32r matmuls (K=128, M=32 fout, N=32
    batch) — single-pass fp32 (~tf32 precision, rel err ~1.5e-4, half the
    PE instruction stream of true fp32 which lowers to LO/HI pairs).
    fp32r requires PSUM base partition 0, so 8 patches pack side-by-side
    along the free axis of one PSUM bank [32, 8x32].
  - ScalarE applies bias+ReLU per PSUM bank into an SBUF staging buffer;
    output stores ride ScalarE's own HWDGE ring LAGGED two groups behind
    the ACT stream, so their dependencies are long complete and they can
    never head-of-line block either the load FIFO or the ACT stream
    (gpsimd/SWDGE stores were tried and added multi-us Q7 drain jitter).
  - Two 4-patch mini-groups at the end halve the final
    load->matmul->ACT->store dependency chain.
Measured: ~62-66 us NEFF exec across runs (~±2 us device jitter), vs a
~48 us pure-traffic roofline at the 358 GB/s per-core HBM wall; ~8.7 us
is fixed engine-boot/Tile-preamble before the first DMA packet can flow,
~4 us is the unavoidable tail (final chain + store completion + Tile
drain barrier).
"""

import numpy as np

N, H, W, C = 32, 128, 128, 32
FH = FW = 4
FOUT = 32
NCORES = 8
PL = 128          # patches per core
NQ = 4            # K-chunks per patch (512 / 128)
KR = 128          # contraction rows per chunk (SBUF partitions)
NG = PL // 4      # 4-patch groups per core

_CACHE = {}


def _build_module(bufs=6, out_splits=8, mm_dtype="float32"):
    from concourse import bacc, tile, mybir

    nc = bacc.Bacc("TRN2", target_bir_lowering=False, debug=False, enable_asserts=False)
    dt = mybir.dt.float32
    mdt = getattr(mybir.dt, mm_dtype)
    # xf packs data and filters: [..., 0:32] = batch cols, [..., 32:64] = fout
    xf = nc.dram_tensor("xf", [KR, PL, NQ, N + FOUT], mdt, kind="ExternalInput").ap()
    bt = nc.dram_tensor("bt", [KR, 1], dt, kind="ExternalInput").ap()
    out = nc.dram_tensor("out", [KR, NG, N], dt, kind="ExternalOutput").ap()

    # Graduated chunk sizes (in patches): small first chunks so the first
    # matmul isn't gated on a full-size load sharing bandwidth round-robin.
    sizes = [2, 2, 4]
    rest = PL - sum(sizes)
    sizes += [8] * (rest // 8)
    assert sum(sizes) == PL
    GSPLIT = NG // out_splits
    relu = mybir.ActivationFunctionType.Relu

    with tile.TileContext(nc) as tc:
        with (
            tc.tile_pool(name="xfpool", bufs=bufs) as xfpool,
            tc.tile_pool(name="psum", bufs=8, space="PSUM") as psum,
            tc.tile_pool(name="misc", bufs=1) as misc,
        ):
            bias_t = misc.tile([KR, 1], dt)
            nc.sync.dma_start(bias_t[:], bt[:])
            staging = misc.tile([KR, NG, N], dt)

            p0 = 0
            for ch, PC in enumerate(sizes):
                xtile = xfpool.tile([KR, PC, NQ, N + FOUT], mdt, tag="xf")
                sl = slice(p0, p0 + PC)
                eng = nc.sync if ch % 2 == 0 else nc.scalar
                eng.dma_start(xtile[:], xf[:, sl, :, :])
                for g in range(PC // 2):
                    gg = (p0 + g * 2) // 4       # psum group id (2 patches/iter)
                    half = (p0 + g * 2) % 4      # 0 or 2: which half of the group
                    if half == 0:
                        ptile = psum.tile([KR, N], dt, tag="ps")
                    for s2 in range(2):
                        s = half + s2
                        p = g * 2 + s2
                        for q in range(NQ):
                            nc.tensor.matmul(
                                ptile[32 * s : 32 * s + 32, :],
                                xtile[:, p, q, N : N + FOUT],  # lhsT [128,32(o)]
                                xtile[:, p, q, 0:N],           # rhs  [128,32(b)]
                                start=(q == 0),
                                stop=(q == NQ - 1),
                                tile_position=(0, 32 * s),
                            )
                    if half == 2:
                        nc.scalar.activation(
                            staging[:, gg, :], ptile[:], relu, bias=bias_t[:]
                        )
                        if (gg + 1) % GSPLIT == 0:
                            osl = slice(gg + 1 - GSPLIT, gg + 1)
                            oeng = nc.sync if gg + 1 == NG else nc.gpsimd
                            oeng.dma_start(out[:, osl, :], staging[:, osl, :])
                p0 += PC
    nc.compile()
    return nc


def _build_module_r(bufs=8):
    """float32r variant: single-pass fp32 matmuls (tf32-ish precision),
    PSUM packing along the free axis (8 patches per bank) since fp32r
    requires dst base partition 0. Half the PE instruction stream of the
    fp32 variant -> fewer IRAM paging stalls."""
    from concourse import bacc, tile, mybir

    nc = bacc.Bacc("TRN2", target_bir_lowering=False, debug=False, enable_asserts=False)
    dt = mybir.dt.float32
    mdt = mybir.dt.float32r
    SG = 8                      # patches per PSUM super-group
    NSG = PL // SG              # 16
    xf = nc.dram_tensor("xf", [KR, PL, NQ, N + FOUT], mdt, kind="ExternalInput").ap()
    bt = nc.dram_tensor("bt", [FOUT, 1], dt, kind="ExternalInput").ap()
    out = nc.dram_tensor("out", [FOUT, PL, N], dt, kind="ExternalOutput").ap()

    # Graduated [2,2,4] head (earliest first matmul; measured tightest
    # variance) and a [4,4] tail that halves the final
    # load->matmul->ACT->store chain.
    sizes = [2, 2, 4] + [8] * ((PL - 16) // 8) + [4, 2, 2]
    assert sum(sizes) == PL
    # PSUM eviction groups: 8-patch banks, except two 4-patch mini-groups
    # at the end so the last matmul->ACT->store chain is half as long.
    groups = [(g * SG, SG) for g in range(NSG - 1)] + [(PL - 8, 4), (PL - 4, 4)]
    gof = {}
    for gi, (s0, gsz) in enumerate(groups):
        for i in range(gsz):
            gof[s0 + i] = (gi, i)
    relu = mybir.ActivationFunctionType.Relu

    with tile.TileContext(nc) as tc:
        with (
            tc.tile_pool(name="xfpool", bufs=bufs) as xfpool,
            tc.tile_pool(name="psum", bufs=6, space="PSUM") as psum,
            tc.tile_pool(name="misc", bufs=1) as misc,
        ):
            # bias rides the scalar ring so it doesn't burn sync's first
            # DMA slot (~0.7 us of stream start).
            bias_t = misc.tile([FOUT, 1], dt)
            nc.scalar.dma_start(bias_t[:], bt[:])
            staging = misc.tile([FOUT, PL, N], dt)

            p0 = 0
            ptile = None
            for ch, PC in enumerate(sizes):
                xtile = xfpool.tile([KR, PC, NQ, N + FOUT], mdt, tag="xf")
                # All loads on sync's single HWDGE FIFO: strictly in-order
                # completions. (Arming chunk 0 on the scalar ring was tried
                # and is bimodal: when sync's big queue gets ahead, chunk 0
                # drains at round-robin half-rate and the in-order PE
                # consumption slips ~8 us.)
                nc.sync.dma_start(xtile[:], xf[:, p0 : p0 + PC, :, :])
                for pl in range(PC):
                    p = p0 + pl
                    gi, i = gof[p]
                    s0, gsz = groups[gi]
                    if i == 0:
                        ptile = psum.tile([FOUT, SG, N], dt, tag="ps")
                    for q in range(NQ):
                        nc.tensor.matmul(
                            ptile[:, i, :],
                            xtile[:, pl, q, N : N + FOUT],  # lhsT [128,32(o)]
                            xtile[:, pl, q, 0:N],           # rhs  [128,32(b)]
                            start=(q == 0),
                            stop=(q == NQ - 1),
                        )
                    if i == gsz - 1:
                        nc.scalar.activation(
                            staging[:, s0 : s0 + gsz, :],
                            ptile[:, :gsz, :],
                            relu,
                            bias=bias_t[:],
                        )
                        # Stores also ride the scalar ring, LAGGED two groups
                        # behind the ACT stream: their ACT dependency is long
                        # complete, so they never stall scalar (and the sync
                        # load ring is untouched). The final two stores are
                        # pure program-order after the last ACT.
                        if gi == len(groups) - 1:
                            a = groups[gi - 2][0]
                            nc.scalar.dma_start(
                                out[:, a:s0, :], staging[:, a:s0, :]
                            )
                            nc.scalar.dma_start(
                                out[:, s0:PL, :], staging[:, s0:PL, :]
                            )
                        elif gi % 2 == 1 and gi >= 3:
                            a = groups[gi - 3][0]
                            b = groups[gi - 1][0]
                            nc.scalar.dma_start(
                                out[:, a:b, :], staging[:, a:b, :]
                            )
                p0 += PC
    nc.compile()
    return nc


def _build_module_h(bufs=8, out_dt="float16"):
    """fp16 variant: inputs marshaled to float16 on host (HBM traffic
    halves vs fp32 — this problem is memory-bound with zero operand
    reuse), matmuls run 1 cycle/row on the PE (vs 4 for fp32r at free
    dim 32 < 256) with fp32 PSUM accumulation. rel err ~2e-4, far under
    the 2e-2 gate. Same stream structure as the fp32r variant."""
    from concourse import bacc, tile, mybir

    nc = bacc.Bacc("TRN2", target_bir_lowering=False, debug=False, enable_asserts=False)
    dt = mybir.dt.float32
    mdt = mybir.dt.float16
    odt = getattr(mybir.dt, out_dt)
    SG = 8                      # patches per PSUM super-group
    NSG = PL // SG              # 16
    xf = nc.dram_tensor("xf", [KR, PL, NQ, N + FOUT], mdt, kind="ExternalInput").ap()
    bt = nc.dram_tensor("bt", [FOUT, 1], dt, kind="ExternalInput").ap()
    out = nc.dram_tensor("out", [FOUT, PL, N], odt, kind="ExternalOutput").ap()

    sizes = [2, 2, 4] + [8] * ((PL - 16) // 8) + [4, 2, 2]
    assert sum(sizes) == PL
    groups = [(g * SG, SG) for g in range(NSG - 1)] + [(PL - 8, 4), (PL - 4, 4)]
    gof = {}
    for gi, (s0, gsz) in enumerate(groups):
        for i in range(gsz):
            gof[s0 + i] = (gi, i)
    relu = mybir.ActivationFunctionType.Relu

    with tile.TileContext(nc) as tc:
        with (
            tc.tile_pool(name="xfpool", bufs=bufs) as xfpool,
            tc.tile_pool(name="psum", bufs=6, space="PSUM") as psum,
            tc.tile_pool(name="misc", bufs=1) as misc,
        ):
            bias_t = misc.tile([FOUT, 1], dt)
            nc.scalar.dma_start(bias_t[:], bt[:])
            staging = misc.tile([FOUT, PL, N], odt)

            p0 = 0
            ptile = None
            for ch, PC in enumerate(sizes):
                xtile = xfpool.tile([KR, PC, NQ, N + FOUT], mdt, tag="xf")
                nc.sync.dma_start(xtile[:], xf[:, p0 : p0 + PC, :, :])
                for pl in range(PC):
                    p = p0 + pl
                    gi, i = gof[p]
                    s0, gsz = groups[gi]
                    if i == 0:
                        ptile = psum.tile([FOUT, SG, N], dt, tag="ps")
                    for q in range(NQ):
                        nc.tensor.matmul(
                            ptile[:, i, :],
                            xtile[:, pl, q, N : N + FOUT],  # lhsT [128,32(o)]
                            xtile[:, pl, q, 0:N],           # rhs  [128,32(b)]
                            start=(q == 0),
                            stop=(q == NQ - 1),
                        )
                    if i == gsz - 1:
                        nc.scalar.activation(
                            staging[:, s0 : s0 + gsz, :],
                            ptile[:, :gsz, :],
                            relu,
                            bias=bias_t[:],
                        )
                        if gi == len(groups) - 1:
                            a = groups[gi - 2][0]
                            nc.scalar.dma_start(
                                out[:, a:s0, :], staging[:, a:s0, :]
                            )
                            nc.scalar.dma_start(
                                out[:, s0:PL, :], staging[:, s0:PL, :]
                            )
                        elif gi % 2 == 1 and gi >= 3:
                            a = groups[gi - 3][0]
                            b = groups[gi - 1][0]
                            nc.scalar.dma_start(
                                out[:, a:b, :], staging[:, a:b, :]
                            )
                p0 += PC
    nc.compile()
    return nc


def _get_module():
    if "nc" not in _CACHE:
        _CACHE["nc"] = _build_module()
    return _CACHE["nc"]


def _marshal(X, filters, bias, dtype=np.float32):
    """Shard + lay out full inputs into per-core device arrays."""
    X = np.ascontiguousarray(np.asarray(X, dtype=np.float32))
    filters = np.ascontiguousarray(np.asarray(filters, dtype=np.float32))
    bias = np.asarray(bias, dtype=np.float32)

    # X: (b, core, pr, i, pc, j, c) -> (core, j, c, pr, pc, i, b)
    xv = X.reshape(N, NCORES, 4, FH, 32, FW, C)
    xt = xv.transpose(1, 5, 6, 2, 4, 3, 0).reshape(NCORES, KR, PL, NQ, N)
    # filters: (core, p, i, j, c, o) -> (core, j, c, p, i, o)
    fv = filters.reshape(NCORES, PL, FH, FW, C, FOUT)
    ft = fv.transpose(0, 3, 4, 1, 2, 5).reshape(NCORES, KR, PL, NQ, FOUT)
    xfa = np.concatenate([xt, ft], axis=4)
    xfa = np.ascontiguousarray(xfa.astype(dtype, copy=False))
    bt = np.ascontiguousarray(np.tile(bias, 4).reshape(KR, 1))
    return xfa, bt


def _assemble(outs):
    """Per-core out [128=(s,o), NG, N] -> full (N, 32, 32, FOUT)."""
    z = np.stack(outs)                                  # (core, (s,o), g, b)
    z = z.reshape(NCORES, 4, FOUT, NG, N)               # (core, s, o, g, b)
    z = z.transpose(4, 0, 3, 1, 2)                      # (b, core, g, s, o)
    z = z.reshape(N, NCORES, PL, FOUT)                  # p_loc = 4*g + s
    z = z.reshape(N, NCORES * 4, 32, FOUT)              # (b, pr_glob, pc, o)
    return np.ascontiguousarray(z)


def _assemble_r(outs):
    """Per-core out [FOUT, PL, N] -> full (N, 32, 32, FOUT)."""
    z = np.stack(outs)                                  # (core, o, p, b)
    z = z.transpose(3, 0, 2, 1)                         # (b, core, p, o)
    z = z.reshape(N, 32, 32, FOUT)
    return np.ascontiguousarray(z.astype(np.float32, copy=False))


LAST_RESULT = None
VARIANT = "fp16"


def kernel(X, filters, bias):
    global LAST_RESULT
    from concourse import bass_utils
    from concourse.bass_utils import run_bass_kernel_spmd

    # If tracing is enabled in the environment, keep the artifact upload
    # local so a missing bucket can't fail the run.
    bass_utils.upload_artifacts = lambda tmpdir: f"local://{tmpdir}"

    if "nc" not in _CACHE:
        _CACHE["nc"] = {
            "fp16": _build_module_h,
            "fp32r": _build_module_r,
            "fp32": _build_module,
        }[VARIANT]()
    nc = _CACHE["nc"]
    xfa, bt = _marshal(
        X, filters, bias, dtype=np.float16 if VARIANT == "fp16" else np.float32
    )
    if VARIANT in ("fp32r", "fp16"):
        bt = np.ascontiguousarray(bt[:FOUT])
    in_maps = [{"xf": xfa[k], "bt": bt} for k in range(NCORES)]
    res = run_bass_kernel_spmd(nc, in_maps, core_ids=list(range(NCORES)))
    LAST_RESULT = res
    outs = [res.results[k]["out"] for k in range(NCORES)]
    return _assemble_r(outs) if VARIANT in ("fp32r", "fp16") else _assemble(outs)

